# revision 2
# baseline (speedup 1.0000x reference)
"""CoverTreeLoss kernel for 8x Trainium2 NeuronCores (Bass/Tile).

Math (matches the reference):
    anc_sum = segment_sum(weights[ancestor_idx], segment_id, T)   # [T, H]
    aw      = weights[:T] + anc_sum                               # [T, H]
    logits  = x @ aw.T                                            # [B, T]
    loss    = mean_b( logsumexp_t(logits[b]) - logits[b, y[b]] )

Distribution: class/vocab-parallel. Core c owns classes [c*1250, (c+1)*1250),
padded to 1280. On device per core:
  1. indirect-DMA gathers (one row per partition per call) pull the 6
     ancestor slots per class (zero-row padded) from the bf16 weight table;
     a direct DMA pulls the leaf rows,
  2. DVE adds fold the 7 slots -> aw rows (the segment reduction),
  3. PE transposes aw -> awT (hidden dim on partitions),
  4. 512-deep matmul x @ awT -> logits in PSUM (fp32 accumulate),
  5. ACT computes exp(logits) with a fused per-row accumulate -> softmax
     normalizer partials; DVE evacuates logits to SBUF; DMA to DRAM.
Host combines the 8 logits shards and the per-core sum-exp partials
(subtracting the exactly-1.0 contribution of each pad column) into the loss.
"""

import sys

if "/opt/trn_rl_repo" not in sys.path:
    sys.path.insert(0, "/opt/trn_rl_repo")

from contextlib import ExitStack

import ml_dtypes
import numpy as np

B = 4096
H = 512
T = 10000  # leaf classes
C = 16384  # total tree nodes
N_CORES = 8
TC = T // N_CORES  # 1250 classes per core
TPAD = 1280        # padded to 10 tiles of 128
NJ = TPAD // 128   # 10
ANC = 6            # ancestor slots per class
SLOTS = ANC + 1    # + 1 leaf slot
ZERO_ROW = C       # index of the appended all-zero row
KC = H // 128      # 4 contraction chunks
N_CHUNKS = [(0, 512), (512, 512), (1024, 256)]
BG = 4             # b-tiles per output DMA group
JGROUPS = [(0, 1, 2, 3), (4, 5, 6, 7), (8, 9)]

BF16 = ml_dtypes.bfloat16

_COMPILED = {}


def _emit_prologue(nc, tc, ctx, dt, bass, mmdt):
    """Gather + segment-fold + transpose; returns (G, awt_sb, xt/idx handles)."""
    const = ctx.enter_context(tc.tile_pool(name="const", bufs=1))
    gpool = ctx.enter_context(tc.tile_pool(name="gather", bufs=1))
    awt_pool = ctx.enter_context(tc.tile_pool(name="awt", bufs=1))
    ps_awt = ctx.enter_context(tc.tile_pool(name="psawt", bufs=2, space="PSUM"))

    wg_d = nc.dram_tensor("wg", [C + 1, H], mmdt, kind="ExternalInput")
    wl_d = nc.dram_tensor("wl", [128, NJ, H], mmdt, kind="ExternalInput")
    idx_d = nc.dram_tensor("idx", [128, NJ * ANC], dt.int32, kind="ExternalInput")
    id_d = nc.dram_tensor("ident", [128, 128], mmdt, kind="ExternalInput")

    idx_sb = const.tile([128, NJ * ANC], dt.int32)
    nc.sync.dma_start(idx_sb[:], idx_d.ap()[:])
    id_sb = const.tile([128, 128], mmdt)
    nc.sync.dma_start(id_sb[:], id_d.ap()[:])

    G = gpool.tile([128, NJ * SLOTS, H], mmdt)
    awt_sb = awt_pool.tile([128, KC, TPAD], mmdt)

    for j in range(NJ):
        s0 = j * SLOTS
        for d in range(ANC):
            nc.gpsimd.indirect_dma_start(
                out=G[:, s0 + d, :],
                out_offset=None,
                in_=wg_d.ap()[:, :],
                in_offset=bass.IndirectOffsetOnAxis(
                    ap=idx_sb[:, j * ANC + d:j * ANC + d + 1], axis=0),
            )
        # leaf rows (zero-padded on host) via direct DMA
        nc.sync.dma_start(G[:, s0 + ANC, :], wl_d.ap()[:, j, :])
        # fold 7 slots into slot s0
        nc.vector.tensor_add(out=G[:, s0:s0 + 3, :], in0=G[:, s0:s0 + 3, :],
                             in1=G[:, s0 + 3:s0 + 6, :])
        nc.vector.tensor_add(out=G[:, s0, :], in0=G[:, s0, :],
                             in1=G[:, s0 + 1, :])
        nc.vector.tensor_add(out=G[:, s0, :], in0=G[:, s0, :],
                             in1=G[:, s0 + 2, :])
        nc.vector.tensor_add(out=G[:, s0, :], in0=G[:, s0, :],
                             in1=G[:, s0 + 6, :])

    for k in range(KC):
        for jg, jlist in enumerate(JGROUPS):
            pst = ps_awt.tile([128, 512], mmdt, tag="psawt")
            for gi, j in enumerate(jlist):
                nc.tensor.transpose(
                    pst[:, gi * 128:(gi + 1) * 128],
                    G[:, j * SLOTS, k * 128:(k + 1) * 128],
                    id_sb[:],
                )
            w = len(jlist) * 128
            nc.vector.tensor_copy(
                out=awt_sb[:, k, jg * 512:jg * 512 + w], in_=pst[:, :w])

    return G, awt_sb, const


def _build_module(mm_dtype, debug_dump=False):
    import concourse.bacc as bacc
    import concourse.bass as bass
    import concourse.mybir as mybir
    import concourse.tile as tile

    dt = mybir.dt
    mmdt = {"bf16": dt.bfloat16, "f32r": dt.float32r, "f32": dt.float32}[mm_dtype]

    nc = bacc.Bacc("TRN2", target_bir_lowering=False, debug=False,
                   num_devices=N_CORES)

    xt_d = nc.dram_tensor("xt", [128, KC, B], mmdt, kind="ExternalInput")
    lg_d = nc.dram_tensor("logits", [B, TC], dt.float32, kind="ExternalOutput")
    st_d = nc.dram_tensor("stats", [128, B // 128], dt.float32,
                          kind="ExternalOutput")
    if debug_dump:
        gf_d = nc.dram_tensor("gfold", [128, NJ, H], mmdt, kind="ExternalOutput")
        awt_d = nc.dram_tensor("awt", [128, KC, TPAD], mmdt,
                               kind="ExternalOutput")

    with tile.TileContext(nc) as tc, ExitStack() as ctx:
        G, awt_sb, const = _emit_prologue(nc, tc, ctx, dt, bass, mmdt)

        lsb_pool = ctx.enter_context(tc.tile_pool(name="lsb", bufs=2))
        scr_pool = ctx.enter_context(tc.tile_pool(name="scr", bufs=2))
        ps_log = ctx.enter_context(tc.tile_pool(name="pslog", bufs=2,
                                                space="PSUM"))

        xt_sb = const.tile([128, KC, B], mmdt)
        nc.sync.dma_start(xt_sb[:], xt_d.ap()[:])
        st_sb = const.tile([128, B // 128], dt.float32)

        if debug_dump:
            for j in range(NJ):
                nc.sync.dma_start(gf_d.ap()[:, j, :], G[:, j * SLOTS, :])
            nc.sync.dma_start(awt_d.ap()[:], awt_sb[:])

        lg_view = lg_d.ap().rearrange("(a g p) t -> p a g t", p=128, g=BG)
        n_btiles = B // 128
        for ig in range(n_btiles // BG):
            lsb = lsb_pool.tile([128, BG, TPAD], dt.float32, tag="lsb")
            for g in range(BG):
                i = ig * BG + g
                ps = ps_log.tile([128, TPAD], dt.float32, tag="pslog")
                for (ns, nw) in N_CHUNKS:
                    for k in range(KC):
                        nc.tensor.matmul(
                            ps[:, ns:ns + nw],
                            lhsT=xt_sb[:, k, i * 128:(i + 1) * 128],
                            rhs=awt_sb[:, k, ns:ns + nw],
                            start=(k == 0),
                            stop=(k == KC - 1),
                        )
                scr = scr_pool.tile([128, TPAD], dt.bfloat16, tag="scr")
                nc.scalar.activation(scr[:], ps[:],
                                     mybir.ActivationFunctionType.Exp,
                                     accum_out=st_sb[:, i:i + 1])
                nc.vector.tensor_copy(out=lsb[:, g, :TC], in_=ps[:, :TC])
            nc.sync.dma_start(lg_view[:, ig], lsb[:, :, :TC])
        nc.sync.dma_start(st_d.ap()[:], st_sb[:])

    nc.compile()
    return nc


def _get_module(mm_dtype="bf16", debug_dump=False):
    key = (mm_dtype, debug_dump)
    if key not in _COMPILED:
        _COMPILED[key] = _build_module(mm_dtype, debug_dump)
    return _COMPILED[key]


def _prep_inputs(x, weights, ancestor_idx, segment_id, np_mmdt):
    """Host-side input shaping; returns per-core input maps."""
    x = np.asarray(x, dtype=np.float32)
    weights = np.asarray(weights, dtype=np.float32)
    anc = np.asarray(ancestor_idx, dtype=np.int64)
    seg = np.asarray(segment_id, dtype=np.int64)

    # x^T wrapped: xt[p, k, b] = x[b, k*128+p]
    xt = np.ascontiguousarray(
        x.T.reshape(KC, 128, B).transpose(1, 0, 2)).astype(np_mmdt)
    # weight table + zero row
    wg = np.concatenate([weights, np.zeros((1, H), np.float32)], axis=0)
    wg = np.ascontiguousarray(wg).astype(np_mmdt)
    ident = np.eye(128, dtype=np_mmdt)

    # per-class ancestor table [T, ANC], zero-row padded
    order = np.argsort(seg, kind="stable")
    anc_s = anc[order]
    counts = np.bincount(seg[order], minlength=T).astype(np.int64)
    offs = np.zeros(T + 1, np.int64)
    np.cumsum(counts, out=offs[1:])
    anc_tab = np.full((T, ANC), ZERO_ROW, np.int64)
    max_d = int(counts.max()) if counts.size else 0
    assert max_d <= ANC, f"class depth {max_d} exceeds slot budget"
    for d in range(max_d):
        m = counts > d
        anc_tab[m, d] = anc_s[offs[:-1][m] + d]

    in_maps = []
    for c in range(N_CORES):
        pad_tab = np.full((TPAD, ANC), ZERO_ROW, np.int64)
        pad_tab[:TC] = anc_tab[c * TC:(c + 1) * TC]
        # device layout: idx[p, j*ANC + d] = pad_tab[j*128+p, d]
        idx_dev = np.ascontiguousarray(
            pad_tab.reshape(NJ, 128, ANC).transpose(1, 0, 2)
        ).reshape(128, NJ * ANC).astype(np.int32)
        # leaf rows, wrapped + zero-padded: wl[p, j, :] = weights[c*TC + j*128+p]
        wl = np.zeros((128, NJ, H), np.float32)
        leaf = np.zeros((TPAD, H), np.float32)
        leaf[:TC] = weights[c * TC:(c + 1) * TC]
        wl[:, :, :] = leaf.reshape(NJ, 128, H).transpose(1, 0, 2)
        in_maps.append({"xt": xt, "wg": wg, "idx": idx_dev,
                        "wl": wl.astype(np_mmdt), "ident": ident})
    return in_maps


def _run(inputs, mm_dtype="bf16", trace=False, trace_kwargs=None,
         debug_dump=False):
    from concourse.bass_utils import run_bass_kernel_spmd

    nc = _get_module(mm_dtype, debug_dump)
    np_mmdt = {"bf16": BF16, "f32r": np.float32, "f32": np.float32}[mm_dtype]
    in_maps = _prep_inputs(inputs["x"], inputs["weights"],
                           inputs["ancestor_idx"], inputs["segment_id"], np_mmdt)
    kw = {}
    if trace:
        kw = dict(trace=True, trace_cores=[0], **(trace_kwargs or {}))
    res = run_bass_kernel_spmd(nc, in_maps, core_ids=list(range(N_CORES)), **kw)
    return res


def _assemble(results, y):
    y = np.asarray(y).astype(np.int64)
    logits = np.concatenate([r["logits"] for r in results], axis=1)
    # stats[p, i] = sum_t exp(logits[i*128+p, t_core]) incl. TPAD-TC pad
    # columns, each contributing exp(0) = 1 exactly.
    sumexp = np.zeros(B, np.float64)
    for r in results:
        sumexp += (r["stats"].astype(np.float64).T.ravel() - float(TPAD - TC))
    lse = np.log(sumexp)
    logit_y = logits[np.arange(B), y]
    loss = np.float32(np.mean(lse - logit_y.astype(np.float64)))
    return loss, logits


def kernel(x, y, weights, ancestor_idx, segment_id):
    res = _run({"x": x, "weights": weights, "ancestor_idx": ancestor_idx,
                "segment_id": segment_id}, mm_dtype="bf16")
    return _assemble(res.results, y)


# revision 4
# speedup vs baseline: 1.0415x; 1.0415x over previous
"""CoverTreeLoss kernel for 8x Trainium2 NeuronCores (Bass/Tile).

Math (matches the reference):
    anc_sum = segment_sum(weights[ancestor_idx], segment_id, T)   # [T, H]
    aw      = weights[:T] + anc_sum                               # [T, H]
    logits  = x @ aw.T                                            # [B, T]
    loss    = mean_b( logsumexp_t(logits[b]) - logits[b, y[b]] )

Distribution: class/vocab-parallel. Core c owns classes [c*1250, (c+1)*1250),
padded to 1280. On device per core:
  1. indirect-DMA gathers (one row per partition per call) pull the 6
     ancestor slots per class (zero-row padded) from the bf16 weight table;
     a direct DMA pulls the leaf rows,
  2. DVE adds fold the 7 slots -> aw rows (the segment reduction),
  3. PE transposes aw -> awT (hidden dim on partitions),
  4. 512-deep matmul x @ awT -> logits in PSUM (fp32 accumulate),
  5. ACT computes exp(logits) with a fused per-row accumulate -> softmax
     normalizer partials; DVE evacuates logits to SBUF; DMA to DRAM.
Host combines the 8 logits shards and the per-core sum-exp partials
(subtracting the exactly-1.0 contribution of each pad column) into the loss.
"""

import sys

if "/opt/trn_rl_repo" not in sys.path:
    sys.path.insert(0, "/opt/trn_rl_repo")

from contextlib import ExitStack

import ml_dtypes
import numpy as np

B = 4096
H = 512
T = 10000  # leaf classes
C = 16384  # total tree nodes
N_CORES = 8
TC = T // N_CORES  # 1250 classes per core
TPAD = 1280        # padded to 10 tiles of 128
NJ = TPAD // 128   # 10
ANC = 6            # ancestor slots per class
SLOTS = ANC + 1    # + 1 leaf slot
ZERO_ROW = C       # index of the appended all-zero row
KC = H // 128      # 4 contraction chunks
N_CHUNKS = [(0, 512), (512, 512), (1024, 256)]
BG = 4             # b-tiles per output DMA group
JGROUPS = [(0, 1, 2, 3), (4, 5, 6, 7), (8, 9)]

BF16 = ml_dtypes.bfloat16

_COMPILED = {}


NIDX16 = SLOTS * 128 // 16  # int16 idx columns per j-group


def _emit_prologue(nc, tc, ctx, dt, bass, mmdt):
    """Gather + segment-fold + transpose; returns (G, awt_sb, xt/idx handles)."""
    from concourse.library_config import mlp
    from concourse.tile import add_dep_helper

    const = ctx.enter_context(tc.tile_pool(name="const", bufs=1))
    gpool = ctx.enter_context(tc.tile_pool(name="gather", bufs=1))
    awt_pool = ctx.enter_context(tc.tile_pool(name="awt", bufs=1))
    ps_awt = ctx.enter_context(tc.tile_pool(name="psawt", bufs=2, space="PSUM"))

    wg_d = nc.dram_tensor("wg", [C + 1, H], mmdt, kind="ExternalInput")
    idx_d = nc.dram_tensor("idx", [128, NJ * NIDX16], dt.int16,
                           kind="ExternalInput")
    id_d = nc.dram_tensor("ident", [128, 128], mmdt, kind="ExternalInput")

    idx_sb = const.tile([128, NJ * NIDX16], dt.int16)
    nc.sync.dma_start(idx_sb[:], idx_d.ap()[:])
    id_sb = const.tile([128, 128], mmdt)
    nc.sync.dma_start(id_sb[:], id_d.ap()[:])

    G = gpool.tile([128, NJ * SLOTS, H], mmdt)
    awt_sb = awt_pool.tile([128, KC, TPAD], mmdt)

    lib_inst = nc.gpsimd.load_library(mlp)

    for j in range(NJ):
        s0 = j * SLOTS
        g = nc.gpsimd.dma_gather(
            out_ap=G[:, s0:s0 + SLOTS, :],
            in_ap=wg_d.ap()[:, :],
            idxs_ap=idx_sb[:, j * NIDX16:(j + 1) * NIDX16],
            num_idxs=SLOTS * 128,
            num_idxs_reg=SLOTS * 128,
            elem_size=H,
        )
        add_dep_helper(g.ins, lib_inst.ins, sync=False,
                       reason="gpsimd library load precedes dma_gather")
        # fold 7 slots into slot s0
        nc.vector.tensor_add(out=G[:, s0:s0 + 3, :], in0=G[:, s0:s0 + 3, :],
                             in1=G[:, s0 + 3:s0 + 6, :])
        nc.vector.tensor_add(out=G[:, s0, :], in0=G[:, s0, :],
                             in1=G[:, s0 + 1, :])
        nc.vector.tensor_add(out=G[:, s0, :], in0=G[:, s0, :],
                             in1=G[:, s0 + 2, :])
        nc.vector.tensor_add(out=G[:, s0, :], in0=G[:, s0, :],
                             in1=G[:, s0 + 6, :])

    for k in range(KC):
        for jg, jlist in enumerate(JGROUPS):
            pst = ps_awt.tile([128, 512], mmdt, tag="psawt")
            for gi, j in enumerate(jlist):
                nc.tensor.transpose(
                    pst[:, gi * 128:(gi + 1) * 128],
                    G[:, j * SLOTS, k * 128:(k + 1) * 128],
                    id_sb[:],
                )
            w = len(jlist) * 128
            nc.vector.tensor_copy(
                out=awt_sb[:, k, jg * 512:jg * 512 + w], in_=pst[:, :w])

    return G, awt_sb, const


def _build_module(mm_dtype, debug_dump=False):
    import concourse.bacc as bacc
    import concourse.bass as bass
    import concourse.mybir as mybir
    import concourse.tile as tile

    dt = mybir.dt
    mmdt = {"bf16": dt.bfloat16, "f32r": dt.float32r, "f32": dt.float32}[mm_dtype]

    nc = bacc.Bacc("TRN2", target_bir_lowering=False, debug=False,
                   num_devices=N_CORES)

    xt_d = nc.dram_tensor("xt", [128, KC, B], mmdt, kind="ExternalInput")
    lg_d = nc.dram_tensor("logits", [B, TC], dt.float32, kind="ExternalOutput")
    st_d = nc.dram_tensor("stats", [128, B // 128], dt.float32,
                          kind="ExternalOutput")
    if debug_dump:
        gf_d = nc.dram_tensor("gfold", [128, NJ, H], mmdt, kind="ExternalOutput")
        awt_d = nc.dram_tensor("awt", [128, KC, TPAD], mmdt,
                               kind="ExternalOutput")

    with tile.TileContext(nc) as tc, ExitStack() as ctx:
        G, awt_sb, const = _emit_prologue(nc, tc, ctx, dt, bass, mmdt)

        lsb_pool = ctx.enter_context(tc.tile_pool(name="lsb", bufs=2))
        scr_pool = ctx.enter_context(tc.tile_pool(name="scr", bufs=2))
        ps_log = ctx.enter_context(tc.tile_pool(name="pslog", bufs=2,
                                                space="PSUM"))

        xt_sb = const.tile([128, KC, B], mmdt)
        nc.sync.dma_start(xt_sb[:], xt_d.ap()[:])
        st_sb = const.tile([128, B // 128], dt.float32)

        if debug_dump:
            for j in range(NJ):
                nc.sync.dma_start(gf_d.ap()[:, j, :], G[:, j * SLOTS, :])
            nc.sync.dma_start(awt_d.ap()[:], awt_sb[:])

        lg_view = lg_d.ap().rearrange("(a g p) t -> p a g t", p=128, g=BG)
        n_btiles = B // 128
        for ig in range(n_btiles // BG):
            lsb = lsb_pool.tile([128, BG, TPAD], dt.float32, tag="lsb")
            for g in range(BG):
                i = ig * BG + g
                ps = ps_log.tile([128, TPAD], dt.float32, tag="pslog")
                for (ns, nw) in N_CHUNKS:
                    for k in range(KC):
                        nc.tensor.matmul(
                            ps[:, ns:ns + nw],
                            lhsT=xt_sb[:, k, i * 128:(i + 1) * 128],
                            rhs=awt_sb[:, k, ns:ns + nw],
                            start=(k == 0),
                            stop=(k == KC - 1),
                        )
                scr = scr_pool.tile([128, TPAD], dt.bfloat16, tag="scr")
                nc.scalar.activation(scr[:], ps[:],
                                     mybir.ActivationFunctionType.Exp,
                                     accum_out=st_sb[:, i:i + 1])
                nc.vector.tensor_copy(out=lsb[:, g, :TC], in_=ps[:, :TC])
            nc.sync.dma_start(lg_view[:, ig], lsb[:, :, :TC])
        nc.sync.dma_start(st_d.ap()[:], st_sb[:])

    nc.compile()
    return nc


def _get_module(mm_dtype="bf16", debug_dump=False):
    key = (mm_dtype, debug_dump)
    if key not in _COMPILED:
        _COMPILED[key] = _build_module(mm_dtype, debug_dump)
    return _COMPILED[key]


def _prep_inputs(x, weights, ancestor_idx, segment_id, np_mmdt):
    """Host-side input shaping; returns per-core input maps."""
    x = np.asarray(x, dtype=np.float32)
    weights = np.asarray(weights, dtype=np.float32)
    anc = np.asarray(ancestor_idx, dtype=np.int64)
    seg = np.asarray(segment_id, dtype=np.int64)

    # x^T wrapped: xt[p, k, b] = x[b, k*128+p]
    xt = np.ascontiguousarray(
        x.T.reshape(KC, 128, B).transpose(1, 0, 2)).astype(np_mmdt)
    # weight table + zero row
    wg = np.concatenate([weights, np.zeros((1, H), np.float32)], axis=0)
    wg = np.ascontiguousarray(wg).astype(np_mmdt)
    ident = np.eye(128, dtype=np_mmdt)

    # per-class slot table [T, SLOTS]: ancestors (zero-row padded) + leaf
    order = np.argsort(seg, kind="stable")
    anc_s = anc[order]
    counts = np.bincount(seg[order], minlength=T).astype(np.int64)
    offs = np.zeros(T + 1, np.int64)
    np.cumsum(counts, out=offs[1:])
    slot_tab = np.full((T, SLOTS), ZERO_ROW, np.int64)
    max_d = int(counts.max()) if counts.size else 0
    assert max_d <= ANC, f"class depth {max_d} exceeds slot budget"
    for d in range(max_d):
        m = counts > d
        slot_tab[m, d] = anc_s[offs[:-1][m] + d]
    slot_tab[:, ANC] = np.arange(T)  # leaf row in slot 6

    in_maps = []
    for c in range(N_CORES):
        pad_tab = np.full((TPAD, SLOTS), ZERO_ROW, np.int64)
        pad_tab[:TC] = slot_tab[c * TC:(c + 1) * TC]
        # dma_gather wants, per j-group, a logical index list where entry
        # i = d*128 + p is the row for (class j*128+p, slot d), wrapped as
        # idx16[i % 16, i // 16] over 16 partitions, replicated x8.
        blocks = []
        for j in range(NJ):
            lst = np.ascontiguousarray(
                pad_tab[j * 128:(j + 1) * 128, :].T).ravel()  # i = d*128+p
            blk = lst.reshape(NIDX16, 16).T  # [16, 56]
            blocks.append(blk)
        idx16 = np.concatenate(blocks, axis=1)  # [16, NJ*56]
        idx_dev = np.ascontiguousarray(
            np.tile(idx16, (8, 1))).astype(np.int16)  # [128, NJ*56]
        in_maps.append({"xt": xt, "wg": wg, "idx": idx_dev, "ident": ident})
    return in_maps


def _run(inputs, mm_dtype="bf16", trace=False, trace_kwargs=None,
         debug_dump=False):
    from concourse.bass_utils import run_bass_kernel_spmd

    nc = _get_module(mm_dtype, debug_dump)
    np_mmdt = {"bf16": BF16, "f32r": np.float32, "f32": np.float32}[mm_dtype]
    in_maps = _prep_inputs(inputs["x"], inputs["weights"],
                           inputs["ancestor_idx"], inputs["segment_id"], np_mmdt)
    kw = {}
    if trace:
        kw = dict(trace=True, trace_cores=[0], **(trace_kwargs or {}))
    res = run_bass_kernel_spmd(nc, in_maps, core_ids=list(range(N_CORES)), **kw)
    return res


def _assemble(results, y):
    y = np.asarray(y).astype(np.int64)
    logits = np.concatenate([r["logits"] for r in results], axis=1)
    # stats[p, i] = sum_t exp(logits[i*128+p, t_core]) incl. TPAD-TC pad
    # columns, each contributing exp(0) = 1 exactly.
    sumexp = np.zeros(B, np.float64)
    for r in results:
        sumexp += (r["stats"].astype(np.float64).T.ravel() - float(TPAD - TC))
    lse = np.log(sumexp)
    logit_y = logits[np.arange(B), y]
    loss = np.float32(np.mean(lse - logit_y.astype(np.float64)))
    return loss, logits


def kernel(x, y, weights, ancestor_idx, segment_id):
    res = _run({"x": x, "weights": weights, "ancestor_idx": ancestor_idx,
                "segment_id": segment_id}, mm_dtype="bf16")
    return _assemble(res.results, y)


# revision 10
# speedup vs baseline: 1.1113x; 1.0670x over previous
"""CoverTreeLoss kernel for 8x Trainium2 NeuronCores (Bass/Tile).

Math (matches the reference):
    anc_sum = segment_sum(weights[ancestor_idx], segment_id, T)   # [T, H]
    aw      = weights[:T] + anc_sum                               # [T, H]
    logits  = x @ aw.T                                            # [B, T]
    loss    = mean_b( logsumexp_t(logits[b]) - logits[b, y[b]] )

Distribution: class/vocab-parallel. Core c owns classes [c*1250, (c+1)*1250),
padded to 1280. On device per core:
  1. indirect-DMA gathers (one row per partition per call) pull the 6
     ancestor slots per class (zero-row padded) from the bf16 weight table;
     a direct DMA pulls the leaf rows,
  2. DVE adds fold the 7 slots -> aw rows (the segment reduction),
  3. PE transposes aw -> awT (hidden dim on partitions),
  4. 512-deep matmul x @ awT -> logits in PSUM (fp32 accumulate),
  5. ACT computes exp(logits) with a fused per-row accumulate -> softmax
     normalizer partials; DVE evacuates logits to SBUF; DMA to DRAM.
Host combines the 8 logits shards and the per-core sum-exp partials
(subtracting the exactly-1.0 contribution of each pad column) into the loss.
"""

import sys

if "/opt/trn_rl_repo" not in sys.path:
    sys.path.insert(0, "/opt/trn_rl_repo")

from contextlib import ExitStack

import ml_dtypes
import numpy as np

B = 4096
H = 512
T = 10000  # leaf classes
C = 16384  # total tree nodes
N_CORES = 8
TC = T // N_CORES  # 1250 classes per core
TPAD = 1280        # padded to 10 tiles of 128
NJ = TPAD // 128   # 10
ANC = 6            # ancestor slots per class
SLOTS = ANC + 1    # + 1 leaf slot
ZERO_ROW = C       # index of the appended all-zero row
KC = H // 128      # 4 contraction chunks
BG = 4             # b-tiles per output DMA group
# transpose-copy groups aligned to the two-pass column split at 640
JGROUPS = [(0, 1, 2, 3), (4,), (5, 6, 7), (8, 9)]
# per-pass (column base, matmul chunks, valid width) — pass A covers classes
# 0:640 (j0-4), pass B 640:1280 (j5-9, last 30 are pads)
PASSES = [
    (0, [(0, 512), (512, 128)], 640),
    (640, [(640, 512), (1152, 128)], 610),
]

BF16 = ml_dtypes.bfloat16

_COMPILED = {}


NIDX16 = SLOTS * 128 // 16  # int16 idx columns per j-group


def _emit_prologue(nc, tc, ctx, dt, bass, mmdt):
    """Gather + segment-fold + transpose; returns (G, awt_sb, xt/idx handles)."""
    from concourse.library_config import mlp
    from concourse.tile import add_dep_helper

    const = ctx.enter_context(tc.tile_pool(name="const", bufs=1))
    gpool = ctx.enter_context(tc.tile_pool(name="gather", bufs=1))
    awt_pool = ctx.enter_context(tc.tile_pool(name="awt", bufs=1))
    ps_awt = ctx.enter_context(tc.tile_pool(name="psawt", bufs=2, space="PSUM"))

    wg_d = nc.dram_tensor("wg", [C + 1, H], mmdt, kind="ExternalInput")
    idx_d = nc.dram_tensor("idx", [128, NJ * NIDX16], dt.int16,
                           kind="ExternalInput")
    id_d = nc.dram_tensor("ident", [128, 128], mmdt, kind="ExternalInput")

    idx_sb = const.tile([128, NJ * NIDX16], dt.int16)
    nc.sync.dma_start(idx_sb[:], idx_d.ap()[:])
    id_sb = const.tile([128, 128], mmdt)
    nc.sync.dma_start(id_sb[:], id_d.ap()[:])

    G = gpool.tile([128, NJ * SLOTS, H], mmdt)
    awt_sb = awt_pool.tile([128, KC, TPAD], mmdt)

    lib_inst = nc.gpsimd.load_library(mlp)

    for j in range(NJ):
        s0 = j * SLOTS
        g = nc.gpsimd.dma_gather(
            out_ap=G[:, s0:s0 + SLOTS, :],
            in_ap=wg_d.ap()[:, :],
            idxs_ap=idx_sb[:, j * NIDX16:(j + 1) * NIDX16],
            num_idxs=SLOTS * 128,
            num_idxs_reg=SLOTS * 128,
            elem_size=H,
        )
        add_dep_helper(g.ins, lib_inst.ins, sync=False,
                       reason="gpsimd library load precedes dma_gather")
        # fold 7 slots into slot s0
        nc.vector.tensor_add(out=G[:, s0:s0 + 3, :], in0=G[:, s0:s0 + 3, :],
                             in1=G[:, s0 + 3:s0 + 6, :])
        nc.vector.tensor_add(out=G[:, s0, :], in0=G[:, s0, :],
                             in1=G[:, s0 + 1, :])
        nc.vector.tensor_add(out=G[:, s0, :], in0=G[:, s0, :],
                             in1=G[:, s0 + 2, :])
        nc.vector.tensor_add(out=G[:, s0, :], in0=G[:, s0, :],
                             in1=G[:, s0 + 6, :])

    for k in range(KC):
        col = 0
        for jlist in JGROUPS:
            pst = ps_awt.tile([128, 512], mmdt, tag="psawt")
            for gi, j in enumerate(jlist):
                nc.tensor.transpose(
                    pst[:, gi * 128:(gi + 1) * 128],
                    G[:, j * SLOTS, k * 128:(k + 1) * 128],
                    id_sb[:],
                )
            w = len(jlist) * 128
            nc.vector.tensor_copy(
                out=awt_sb[:, k, col:col + w], in_=pst[:, :w])
            col += w

    return G, awt_sb, const


def _build_module(mm_dtype, debug_dump=False):
    import concourse.bacc as bacc
    import concourse.bass as bass
    import concourse.mybir as mybir
    import concourse.tile as tile

    dt = mybir.dt
    mmdt = {"bf16": dt.bfloat16, "f32r": dt.float32r, "f32": dt.float32}[mm_dtype]

    nc = bacc.Bacc("TRN2", target_bir_lowering=False, debug=False,
                   num_devices=N_CORES)

    xt_d = nc.dram_tensor("xt", [128, KC, B], mmdt, kind="ExternalInput")
    lg_d = nc.dram_tensor("logits", [B, TC], dt.float32, kind="ExternalOutput")
    st_d = nc.dram_tensor("stats", [128, 2 * (B // 128)], dt.float32,
                          kind="ExternalOutput")
    if debug_dump:
        gf_d = nc.dram_tensor("gfold", [128, NJ, H], mmdt, kind="ExternalOutput")
        awt_d = nc.dram_tensor("awt", [128, KC, TPAD], mmdt,
                               kind="ExternalOutput")

    with tile.TileContext(nc) as tc, ExitStack() as ctx:
        G, awt_sb, const = _emit_prologue(nc, tc, ctx, dt, bass, mmdt)

        lsb_pool = ctx.enter_context(tc.tile_pool(name="lsb", bufs=2))
        scr_pool = ctx.enter_context(tc.tile_pool(name="scr", bufs=2))
        ps_log = ctx.enter_context(tc.tile_pool(name="pslog", bufs=3,
                                                space="PSUM"))

        xt_sb = const.tile([128, KC, B], mmdt)
        nc.sync.dma_start(xt_sb[:], xt_d.ap()[:])
        st_sb = const.tile([128, 2 * (B // 128)], dt.float32)

        if debug_dump:
            for j in range(NJ):
                nc.sync.dma_start(gf_d.ap()[:, j, :], G[:, j * SLOTS, :])
            nc.sync.dma_start(awt_d.ap()[:], awt_sb[:])

        n_btiles = B // 128
        for pi, (base, chunks, valid) in enumerate(PASSES):
            lg_view = lg_d.ap()[:, base:base + valid].rearrange(
                "(a g p) t -> p a g t", p=128, g=BG)
            for ig in range(n_btiles // BG):
                lsb = lsb_pool.tile([128, BG, 640], dt.float32, tag="lsb")
                for g in range(BG):
                    i = ig * BG + g
                    ps = ps_log.tile([128, 640], dt.float32, tag="pslog")
                    for (ns, nw) in chunks:
                        for k in range(KC):
                            nc.tensor.matmul(
                                ps[:, ns - base:ns - base + nw],
                                lhsT=xt_sb[:, k, i * 128:(i + 1) * 128],
                                rhs=awt_sb[:, k, ns:ns + nw],
                                start=(k == 0),
                                stop=(k == KC - 1),
                            )
                    scr = scr_pool.tile([128, 640], dt.bfloat16, tag="scr")
                    nc.scalar.activation(scr[:, :valid], ps[:, :valid],
                                         mybir.ActivationFunctionType.Exp,
                                         accum_out=st_sb[:, pi * n_btiles + i:
                                                         pi * n_btiles + i + 1])
                    nc.vector.tensor_copy(out=lsb[:, g, :valid],
                                          in_=ps[:, :valid])
                nc.sync.dma_start(lg_view[:, ig], lsb[:, :, :valid])
        nc.sync.dma_start(st_d.ap()[:], st_sb[:])

    nc.compile()
    return nc


def _get_module(mm_dtype="bf16", debug_dump=False):
    key = (mm_dtype, debug_dump)
    if key not in _COMPILED:
        _COMPILED[key] = _build_module(mm_dtype, debug_dump)
    return _COMPILED[key]


def _prep_inputs(x, weights, ancestor_idx, segment_id, np_mmdt):
    """Host-side input shaping; returns per-core input maps."""
    x = np.asarray(x, dtype=np.float32)
    weights = np.asarray(weights, dtype=np.float32)
    anc = np.asarray(ancestor_idx, dtype=np.int64)
    seg = np.asarray(segment_id, dtype=np.int64)

    # x^T wrapped: xt[p, k, b] = x[b, k*128+p]
    xt = np.ascontiguousarray(
        x.T.reshape(KC, 128, B).transpose(1, 0, 2)).astype(np_mmdt)
    # weight table + zero row
    wg = np.concatenate([weights, np.zeros((1, H), np.float32)], axis=0)
    wg = np.ascontiguousarray(wg).astype(np_mmdt)
    ident = np.eye(128, dtype=np_mmdt)

    # per-class slot table [T, SLOTS]: ancestors (zero-row padded) + leaf
    order = np.argsort(seg, kind="stable")
    anc_s = anc[order]
    counts = np.bincount(seg[order], minlength=T).astype(np.int64)
    offs = np.zeros(T + 1, np.int64)
    np.cumsum(counts, out=offs[1:])
    slot_tab = np.full((T, SLOTS), ZERO_ROW, np.int64)
    max_d = int(counts.max()) if counts.size else 0
    assert max_d <= ANC, f"class depth {max_d} exceeds slot budget"
    for d in range(max_d):
        m = counts > d
        slot_tab[m, d] = anc_s[offs[:-1][m] + d]
    slot_tab[:, ANC] = np.arange(T)  # leaf row in slot 6

    in_maps = []
    for c in range(N_CORES):
        pad_tab = np.full((TPAD, SLOTS), ZERO_ROW, np.int64)
        pad_tab[:TC] = slot_tab[c * TC:(c + 1) * TC]
        # dma_gather wants, per j-group, a logical index list where entry
        # i = d*128 + p is the row for (class j*128+p, slot d), wrapped as
        # idx16[i % 16, i // 16] over 16 partitions, replicated x8.
        blocks = []
        for j in range(NJ):
            lst = np.ascontiguousarray(
                pad_tab[j * 128:(j + 1) * 128, :].T).ravel()  # i = d*128+p
            blk = lst.reshape(NIDX16, 16).T  # [16, 56]
            blocks.append(blk)
        idx16 = np.concatenate(blocks, axis=1)  # [16, NJ*56]
        idx_dev = np.ascontiguousarray(
            np.tile(idx16, (8, 1))).astype(np.int16)  # [128, NJ*56]
        in_maps.append({"xt": xt, "wg": wg, "idx": idx_dev, "ident": ident})
    return in_maps


def _run(inputs, mm_dtype="bf16", trace=False, trace_kwargs=None,
         debug_dump=False):
    from concourse.bass_utils import run_bass_kernel_spmd

    nc = _get_module(mm_dtype, debug_dump)
    np_mmdt = {"bf16": BF16, "f32r": np.float32, "f32": np.float32}[mm_dtype]
    in_maps = _prep_inputs(inputs["x"], inputs["weights"],
                           inputs["ancestor_idx"], inputs["segment_id"], np_mmdt)
    kw = {}
    if trace:
        kw = dict(trace=True, trace_cores=[0], **(trace_kwargs or {}))
    res = run_bass_kernel_spmd(nc, in_maps, core_ids=list(range(N_CORES)), **kw)
    return res


def _assemble(results, y):
    y = np.asarray(y).astype(np.int64)
    logits = np.concatenate([r["logits"] for r in results], axis=1)
    # stats[p, i] / stats[p, 32+i] = pass-A / pass-B sum_t exp(logits[b, t])
    # for row b = i*128+p; pad columns are excluded on device.
    nbt = B // 128
    sumexp = np.zeros(B, np.float64)
    for r in results:
        st = r["stats"].astype(np.float64)
        sumexp += st[:, :nbt].T.ravel() + st[:, nbt:].T.ravel()
    lse = np.log(sumexp)
    logit_y = logits[np.arange(B), y]
    loss = np.float32(np.mean(lse - logit_y.astype(np.float64)))
    return loss, logits


def kernel(x, y, weights, ancestor_idx, segment_id):
    res = _run({"x": x, "weights": weights, "ancestor_idx": ancestor_idx,
                "segment_id": segment_id}, mm_dtype="bf16")
    return _assemble(res.results, y)


# revision 12
# speedup vs baseline: 1.1609x; 1.0447x over previous
"""CoverTreeLoss kernel for 8x Trainium2 NeuronCores (Bass/Tile).

Math (matches the reference):
    anc_sum = segment_sum(weights[ancestor_idx], segment_id, T)   # [T, H]
    aw      = weights[:T] + anc_sum                               # [T, H]
    logits  = x @ aw.T                                            # [B, T]
    loss    = mean_b( logsumexp_t(logits[b]) - logits[b, y[b]] )

Distribution: class/vocab-parallel. Core c owns classes [c*1250, (c+1)*1250),
padded to 1280. On device per core:
  1. indirect-DMA gathers (one row per partition per call) pull the 6
     ancestor slots per class (zero-row padded) from the bf16 weight table;
     a direct DMA pulls the leaf rows,
  2. DVE adds fold the 7 slots -> aw rows (the segment reduction),
  3. PE transposes aw -> awT (hidden dim on partitions),
  4. 512-deep matmul x @ awT -> logits in PSUM (fp32 accumulate),
  5. ACT computes exp(logits) with a fused per-row accumulate -> softmax
     normalizer partials; DVE evacuates logits to SBUF; DMA to DRAM.
Host combines the 8 logits shards and the per-core sum-exp partials
(subtracting the exactly-1.0 contribution of each pad column) into the loss.
"""

import sys

if "/opt/trn_rl_repo" not in sys.path:
    sys.path.insert(0, "/opt/trn_rl_repo")

from contextlib import ExitStack

import ml_dtypes
import numpy as np

B = 4096
H = 512
T = 10000  # leaf classes
C = 16384  # total tree nodes
N_CORES = 8
TC = T // N_CORES  # 1250 classes per core
TPAD = 1280        # padded to 10 tiles of 128
NJ = TPAD // 128   # 10
ANC = 6            # ancestor slots per class
SLOTS = ANC + 1    # + 1 leaf slot
ZERO_ROW = C       # index of the appended all-zero row
KC = H // 128      # 4 contraction chunks
BG = 4             # b-tiles per output DMA group
# transpose-copy groups aligned to the two-pass column split at 640
JGROUPS = [(0, 1, 2, 3), (4,), (5, 6, 7), (8, 9)]
# per-pass (column base, matmul chunks, valid width) — pass A covers classes
# 0:640 (j0-4), pass B 640:1280 (j5-9, last 30 are pads)
PASSES = [
    (0, [(0, 512), (512, 128)], 640),
    (640, [(640, 512), (1152, 128)], 610),
]

BF16 = ml_dtypes.bfloat16

_COMPILED = {}


NIDX16 = SLOTS * 128 // 16  # int16 idx columns per j-group


def _emit_prologue(nc, tc, ctx, dt, bass, mmdt):
    """Gather + segment-fold + transpose; returns (G, awt_sb, xt/idx handles)."""
    from concourse.library_config import mlp
    from concourse.tile import add_dep_helper

    const = ctx.enter_context(tc.tile_pool(name="const", bufs=1))
    gpool = ctx.enter_context(tc.tile_pool(name="gather", bufs=1))
    awt_pool = ctx.enter_context(tc.tile_pool(name="awt", bufs=1))
    ps_awt = ctx.enter_context(tc.tile_pool(name="psawt", bufs=2, space="PSUM"))

    wg_d = nc.dram_tensor("wg", [C + 1, H], mmdt, kind="ExternalInput")
    idx_d = nc.dram_tensor("idx", [128, NJ * NIDX16], dt.int16,
                           kind="ExternalInput")
    id_d = nc.dram_tensor("ident", [128, 128], mmdt, kind="ExternalInput")

    idx_sb = const.tile([128, NJ * NIDX16], dt.int16)
    nc.sync.dma_start(idx_sb[:], idx_d.ap()[:])
    id_sb = const.tile([128, 128], mmdt)
    nc.sync.dma_start(id_sb[:], id_d.ap()[:])

    G = gpool.tile([128, NJ * SLOTS, H], mmdt)
    awt_sb = awt_pool.tile([128, KC, TPAD], mmdt)

    lib_inst = nc.gpsimd.load_library(mlp)

    for j in range(NJ):
        s0 = j * SLOTS
        g = nc.gpsimd.dma_gather(
            out_ap=G[:, s0:s0 + SLOTS, :],
            in_ap=wg_d.ap()[:, :],
            idxs_ap=idx_sb[:, j * NIDX16:(j + 1) * NIDX16],
            num_idxs=SLOTS * 128,
            num_idxs_reg=SLOTS * 128,
            elem_size=H,
        )
        add_dep_helper(g.ins, lib_inst.ins, sync=False,
                       reason="gpsimd library load precedes dma_gather")
        # fold 7 slots into slot s0
        nc.vector.tensor_add(out=G[:, s0:s0 + 3, :], in0=G[:, s0:s0 + 3, :],
                             in1=G[:, s0 + 3:s0 + 6, :])
        nc.vector.tensor_add(out=G[:, s0, :], in0=G[:, s0, :],
                             in1=G[:, s0 + 1, :])
        nc.vector.tensor_add(out=G[:, s0, :], in0=G[:, s0, :],
                             in1=G[:, s0 + 2, :])
        nc.vector.tensor_add(out=G[:, s0, :], in0=G[:, s0, :],
                             in1=G[:, s0 + 6, :])

    col = 0
    for jlist in JGROUPS:
        w = len(jlist) * 128
        for k in range(KC):
            pst = ps_awt.tile([128, 512], mmdt, tag="psawt")
            for gi, j in enumerate(jlist):
                nc.tensor.transpose(
                    pst[:, gi * 128:(gi + 1) * 128],
                    G[:, j * SLOTS, k * 128:(k + 1) * 128],
                    id_sb[:],
                )
            nc.vector.tensor_copy(
                out=awt_sb[:, k, col:col + w], in_=pst[:, :w])
        col += w

    return G, awt_sb, const


def _build_module(mm_dtype, debug_dump=False):
    import concourse.bacc as bacc
    import concourse.bass as bass
    import concourse.mybir as mybir
    import concourse.tile as tile

    dt = mybir.dt
    mmdt = {"bf16": dt.bfloat16, "f32r": dt.float32r, "f32": dt.float32}[mm_dtype]

    nc = bacc.Bacc("TRN2", target_bir_lowering=False, debug=False,
                   num_devices=N_CORES)

    xt_d = nc.dram_tensor("xt", [128, KC, B], mmdt, kind="ExternalInput")
    lg_d = nc.dram_tensor("logits", [B, TC], dt.float32, kind="ExternalOutput")
    st_d = nc.dram_tensor("stats", [128, 2 * (B // 128)], dt.float32,
                          kind="ExternalOutput")
    if debug_dump:
        gf_d = nc.dram_tensor("gfold", [128, NJ, H], mmdt, kind="ExternalOutput")
        awt_d = nc.dram_tensor("awt", [128, KC, TPAD], mmdt,
                               kind="ExternalOutput")

    with tile.TileContext(nc) as tc, ExitStack() as ctx:
        G, awt_sb, const = _emit_prologue(nc, tc, ctx, dt, bass, mmdt)

        lsb_pool = ctx.enter_context(tc.tile_pool(name="lsb", bufs=2))
        scr_pool = ctx.enter_context(tc.tile_pool(name="scr", bufs=2))
        ps_log = ctx.enter_context(tc.tile_pool(name="pslog", bufs=3,
                                                space="PSUM"))

        xt_sb = const.tile([128, KC, B], mmdt)
        nc.sync.dma_start(xt_sb[:], xt_d.ap()[:])
        st_sb = const.tile([128, 2 * (B // 128)], dt.float32)

        if debug_dump:
            for j in range(NJ):
                nc.sync.dma_start(gf_d.ap()[:, j, :], G[:, j * SLOTS, :])
            nc.sync.dma_start(awt_d.ap()[:], awt_sb[:])

        from concourse.tile import add_dep_helper

        n_btiles = B // 128
        prev_mm = None
        for pi, (base, chunks, valid) in enumerate(PASSES):
            lg_view = lg_d.ap()[:, base:base + valid].rearrange(
                "(a g p) t -> p a g t", p=128, g=BG)
            first_mm_of_pass = None
            for ig in range(n_btiles // BG):
                lsb = lsb_pool.tile([128, BG, 640], dt.float32, tag="lsb")
                for g in range(BG):
                    i = ig * BG + g
                    ps = ps_log.tile([128, 640], dt.float32, tag="pslog")
                    for (ns, nw) in chunks:
                        for k in range(KC):
                            m = nc.tensor.matmul(
                                ps[:, ns - base:ns - base + nw],
                                lhsT=xt_sb[:, k, i * 128:(i + 1) * 128],
                                rhs=awt_sb[:, k, ns:ns + nw],
                                start=(k == 0),
                                stop=(k == KC - 1),
                            )
                            if first_mm_of_pass is None:
                                first_mm_of_pass = m
                                if prev_mm is not None:
                                    # keep the in-order PE queue free of
                                    # pass-B heads until pass A is done
                                    add_dep_helper(m.ins, prev_mm.ins,
                                                   sync=False,
                                                   reason="pass phase order")
                            prev_mm = m
                    scr = scr_pool.tile([128, 640], dt.bfloat16, tag="scr")
                    nc.scalar.activation(scr[:, :valid], ps[:, :valid],
                                         mybir.ActivationFunctionType.Exp,
                                         accum_out=st_sb[:, pi * n_btiles + i:
                                                         pi * n_btiles + i + 1])
                    nc.vector.tensor_copy(out=lsb[:, g, :valid],
                                          in_=ps[:, :valid])
                nc.sync.dma_start(lg_view[:, ig], lsb[:, :, :valid])
        nc.sync.dma_start(st_d.ap()[:], st_sb[:])

    nc.compile()
    return nc


def _get_module(mm_dtype="bf16", debug_dump=False):
    key = (mm_dtype, debug_dump)
    if key not in _COMPILED:
        _COMPILED[key] = _build_module(mm_dtype, debug_dump)
    return _COMPILED[key]


def _prep_inputs(x, weights, ancestor_idx, segment_id, np_mmdt):
    """Host-side input shaping; returns per-core input maps."""
    x = np.asarray(x, dtype=np.float32)
    weights = np.asarray(weights, dtype=np.float32)
    anc = np.asarray(ancestor_idx, dtype=np.int64)
    seg = np.asarray(segment_id, dtype=np.int64)

    # x^T wrapped: xt[p, k, b] = x[b, k*128+p]
    xt = np.ascontiguousarray(
        x.T.reshape(KC, 128, B).transpose(1, 0, 2)).astype(np_mmdt)
    # weight table + zero row
    wg = np.concatenate([weights, np.zeros((1, H), np.float32)], axis=0)
    wg = np.ascontiguousarray(wg).astype(np_mmdt)
    ident = np.eye(128, dtype=np_mmdt)

    # per-class slot table [T, SLOTS]: ancestors (zero-row padded) + leaf
    order = np.argsort(seg, kind="stable")
    anc_s = anc[order]
    counts = np.bincount(seg[order], minlength=T).astype(np.int64)
    offs = np.zeros(T + 1, np.int64)
    np.cumsum(counts, out=offs[1:])
    slot_tab = np.full((T, SLOTS), ZERO_ROW, np.int64)
    max_d = int(counts.max()) if counts.size else 0
    assert max_d <= ANC, f"class depth {max_d} exceeds slot budget"
    for d in range(max_d):
        m = counts > d
        slot_tab[m, d] = anc_s[offs[:-1][m] + d]
    slot_tab[:, ANC] = np.arange(T)  # leaf row in slot 6

    in_maps = []
    for c in range(N_CORES):
        pad_tab = np.full((TPAD, SLOTS), ZERO_ROW, np.int64)
        pad_tab[:TC] = slot_tab[c * TC:(c + 1) * TC]
        # dma_gather wants, per j-group, a logical index list where entry
        # i = d*128 + p is the row for (class j*128+p, slot d), wrapped as
        # idx16[i % 16, i // 16] over 16 partitions, replicated x8.
        blocks = []
        for j in range(NJ):
            lst = np.ascontiguousarray(
                pad_tab[j * 128:(j + 1) * 128, :].T).ravel()  # i = d*128+p
            blk = lst.reshape(NIDX16, 16).T  # [16, 56]
            blocks.append(blk)
        idx16 = np.concatenate(blocks, axis=1)  # [16, NJ*56]
        idx_dev = np.ascontiguousarray(
            np.tile(idx16, (8, 1))).astype(np.int16)  # [128, NJ*56]
        in_maps.append({"xt": xt, "wg": wg, "idx": idx_dev, "ident": ident})
    return in_maps


def _run(inputs, mm_dtype="bf16", trace=False, trace_kwargs=None,
         debug_dump=False):
    from concourse.bass_utils import run_bass_kernel_spmd

    nc = _get_module(mm_dtype, debug_dump)
    np_mmdt = {"bf16": BF16, "f32r": np.float32, "f32": np.float32}[mm_dtype]
    in_maps = _prep_inputs(inputs["x"], inputs["weights"],
                           inputs["ancestor_idx"], inputs["segment_id"], np_mmdt)
    kw = {}
    if trace:
        kw = dict(trace=True, trace_cores=[0], **(trace_kwargs or {}))
    res = run_bass_kernel_spmd(nc, in_maps, core_ids=list(range(N_CORES)), **kw)
    return res


def _assemble(results, y):
    y = np.asarray(y).astype(np.int64)
    logits = np.concatenate([r["logits"] for r in results], axis=1)
    # stats[p, i] / stats[p, 32+i] = pass-A / pass-B sum_t exp(logits[b, t])
    # for row b = i*128+p; pad columns are excluded on device.
    nbt = B // 128
    sumexp = np.zeros(B, np.float64)
    for r in results:
        st = r["stats"].astype(np.float64)
        sumexp += st[:, :nbt].T.ravel() + st[:, nbt:].T.ravel()
    lse = np.log(sumexp)
    logit_y = logits[np.arange(B), y]
    loss = np.float32(np.mean(lse - logit_y.astype(np.float64)))
    return loss, logits


def kernel(x, y, weights, ancestor_idx, segment_id):
    res = _run({"x": x, "weights": weights, "ancestor_idx": ancestor_idx,
                "segment_id": segment_id}, mm_dtype="bf16")
    return _assemble(res.results, y)
